# revision 38
# baseline (speedup 1.0000x reference)
"""GNN message-passing encoder (4 layers) on 8 TRN2 NeuronCores via Bass/Tile.

Self-contained: host-side sharding/prep in numpy, SPMD bass program, gather of
full outputs. See test.py for the correctness/timing harness.
"""

import math
import os
import sys
from dataclasses import dataclass, field

import numpy as np

sys.path.insert(0, "/opt/trn_rl_repo")

import ml_dtypes  # noqa: E402

BF16 = ml_dtypes.bfloat16

# ---------------------------------------------------------------- config


@dataclass
class Cfg:
    n_nodes: int = 50000
    nnf: int = 16
    nef: int = 4
    hid: int = 64
    nz: int = 16
    n_cores: int = 8
    chunk: int = 512  # edges per chunk (half of a dual-chunk)
    ln_eps: float = 1e-5
    # derived
    npc: int = 0
    nodes_pad: int = 0
    agg_rows: int = 0
    tab_rows: int = 0
    viewb_off: int = 0
    split: int = 0
    nchunk: int = 0  # chunks per core (even)
    e_pad: int = 0
    ecols: int = 0
    vrows: int = 0  # agg partials table rows
    # (nin, ein, eout, nout) per layer
    layer_dims: tuple = ()

    def finalize(self, max_core_edges):
        assert self.n_nodes % self.n_cores == 0
        self.npc = self.n_nodes // self.n_cores
        self.nodes_pad = ((self.npc + self.chunk - 1) // self.chunk) * self.chunk
        self.agg_rows = self.nodes_pad + 128
        self.tab_rows = self.n_nodes + 2
        self.viewb_off = max(0, self.tab_rows - 32768)
        self.split = min(self.n_nodes, 32767)
        nd = (max_core_edges + 2 * self.chunk - 1) // (2 * self.chunk)
        nd = max(nd, 1)
        self.nchunk = 2 * nd
        self.e_pad = self.nchunk * self.chunk
        self.ecols = self.e_pad // 2
        h = self.hid
        self.layer_dims = (
            (self.nnf, self.nef, h, h),
            (h, h, h, h),
            (h, h, h, h),
            (h, h, 1, self.nz),
        )
        return self


# ---------------------------------------------------------------- host prep


def _wrap_idx(flat, cfg, chunk=None):
    """[n*chunk] int -> [128, n*(chunk//16)] int16, per-chunk wrapped (i -> [i%16, i//16])."""
    ch = chunk or cfg.chunk
    a = np.asarray(flat, np.int64).reshape(-1, ch)
    n = a.shape[0]
    a = a.reshape(n, ch // 16, 16).transpose(0, 2, 1)  # [n,16,ch/16]
    a = a.transpose(1, 0, 2).reshape(16, n * (ch // 16))
    assert a.max() <= 32767 and a.min() >= 0, (a.min(), a.max())
    return np.tile(a, (8, 1)).astype(np.int16)


def _pad_rows(x, width):
    """[n, f] -> [n, width] bf16 (zero pad cols)."""
    n, f = x.shape
    out = np.zeros((n, width), BF16)
    out[:, :f] = x.astype(BF16)
    return out


def host_prep(cfg_base, x, edge_index, edge_attr, params):
    """Shard + build all per-core input arrays. Returns (cfg, in_maps, meta)."""
    x = np.asarray(x, np.float32)
    edge_index = np.asarray(edge_index, np.int64)
    edge_attr = np.asarray(edge_attr, np.float32)
    src, dst = edge_index[0], edge_index[1]
    E = src.shape[0]

    npc = cfg_base.n_nodes // cfg_base.n_cores
    core_of = dst // npc
    perms, counts = [], []
    for c in range(cfg_base.n_cores):
        ids = np.nonzero(core_of == c)[0]
        ids = ids[np.argsort(dst[ids], kind="stable")]
        perms.append(ids)
        counts.append(len(ids))
    cfg = cfg_base
    cfg.finalize(max(counts))

    # ---- weights prep (shared across cores)
    H = cfg.hid
    WL = []  # per-layer dict of arrays
    for li, p in enumerate(params):
        nin, ein, eout, nout = cfg.layer_dims[li]
        pe, pn = p["edge"], p["node"]
        W1 = np.asarray(pe["W1"], np.float32)
        w = {}
        w["Wsrc"] = W1[:nin].astype(BF16)  # [nin, H]
        w["Wdst"] = W1[nin : 2 * nin].astype(BF16)
        w["We"] = W1[2 * nin :].astype(BF16)  # [ein, H]
        b1 = np.asarray(pe["b1"], np.float32)
        w["b1c"] = b1.reshape(H, 1)
        w["gfull"] = np.tile(np.asarray(pe["g"], np.float32), (128, 1))
        w["befull"] = np.tile(np.asarray(pe["be"], np.float32), (128, 1))
        W2 = np.asarray(pe["W2"], np.float32)  # [H, eout]
        w["W2"] = W2.astype(BF16)
        b2 = np.asarray(pe["b2"], np.float32)
        w["b2c"] = b2.reshape(eout, 1)
        # node side
        W1n = np.asarray(pn["W1"], np.float32)
        w["W1nx"] = W1n[:nin].astype(BF16)  # [nin, H]
        W1na = W1n[nin:]  # [eout, H]
        w["Cmat"] = (W2 @ W1na).astype(np.float32)  # [H, H]
        w["vbrow"] = (b2 @ W1na).reshape(1, H).astype(np.float32)
        w["b1nc"] = np.asarray(pn["b1"], np.float32).reshape(H, 1)
        w["gnfull"] = np.tile(np.asarray(pn["g"], np.float32), (128, 1))
        w["benfull"] = np.tile(np.asarray(pn["be"], np.float32), (128, 1))
        w["W2n"] = np.asarray(pn["W2"], np.float32)  # [H, nout]
        w["b2nc"] = np.asarray(pn["b2"], np.float32).reshape(nout, 1)
        WL.append(w)

    # pack f32 consts into one [128, *] tensor, bf16 consts into another
    f32_cols, bf_cols = [], []
    f32_off, bf_off = {}, {}

    def put(cols, off, name, arr, dt):
        a = np.zeros((128, arr.shape[1]), dt)
        a[: arr.shape[0]] = arr.astype(dt)
        off[name] = (sum(c.shape[1] for c in cols), arr.shape[1], arr.shape[0])
        cols.append(a)

    ident = np.eye(128, dtype=np.float32)
    put(f32_cols, f32_off, "idT", ident, np.float32)
    put(bf_cols, bf_off, "idB", ident, BF16)
    put(bf_cols, bf_off, "I64", np.eye(64, dtype=np.float32), BF16)
    put(f32_cols, f32_off, "epsc", np.full((128, 1), cfg.ln_eps, np.float32), np.float32)
    for li, w in enumerate(WL):
        for k, v in w.items():
            if v.dtype == BF16:
                put(bf_cols, bf_off, f"L{li}_{k}", v, BF16)
            else:
                put(f32_cols, f32_off, f"L{li}_{k}", v, np.float32)
    constf = np.concatenate(f32_cols, axis=1)
    constb = np.concatenate(bf_cols, axis=1)

    # ---- global gather table for layer 1 (built from x0)
    x0tab = np.zeros((cfg.tab_rows, 128), BF16)
    x0tab[1 : 1 + cfg.n_nodes] = _pad_rows(x, 128)

    # ---- per-core chunking: whole-dst-segment chunks (<=CHUNK edges, <=64 nodes)
    CH = cfg.chunk
    MAXN = 64
    core_chunks = []  # per core: list of (edge_ids list, node_ids list)
    for c in range(cfg.n_cores):
        ids = perms[c]
        base = c * cfg.npc
        dl = dst[ids] - base
        # group boundaries by dst value
        chunks = []
        cur_e, cur_n = [], []
        i = 0
        Ec = len(ids)
        while i < Ec:
            j = i
            while j < Ec and dl[j] == dl[i]:
                j += 1
            seg = list(range(i, j))
            if cur_e and (len(cur_e) + len(seg) > CH or len(cur_n) >= MAXN):
                chunks.append((cur_e, cur_n))
                cur_e, cur_n = [], []
            # a single segment larger than CH cannot happen (max degree << CH)
            assert len(seg) <= CH
            cur_e += seg
            cur_n.append(int(dl[i]))
            i = j
        if cur_e:
            chunks.append((cur_e, cur_n))
        core_chunks.append(chunks)

    nchunk = max(len(ch) for ch in core_chunks)
    nchunk = ((nchunk + 1) // 2) * 2  # even for dual-chunks
    cfg.nchunk = nchunk
    cfg.e_pad = nchunk * CH
    cfg.ecols = cfg.e_pad // 2
    cfg.vrows = nchunk * MAXN + 1  # partials table rows (+1 zero row)
    assert cfg.vrows - 1 <= 32767

    in_maps = []
    meta = {"perms": perms, "counts": counts, "cfg": cfg, "orders": []}
    for c in range(cfg.n_cores):
        ids = perms[c]
        base = c * cfg.npc
        chunks = core_chunks[c]
        order = np.full(cfg.e_pad, -1, np.int64)  # flat pos -> orig edge id
        s = np.zeros(cfg.e_pad, np.int64)
        ea = np.zeros((cfg.e_pad, cfg.nef), np.float32)
        spT = np.zeros((128, nchunk * 4, MAXN), BF16)
        vslot = np.full(cfg.nodes_pad, cfg.vrows - 1, np.int64)  # default zero row
        for ci, (es, ns) in enumerate(chunks):
            n_rank = {n: r for r, n in enumerate(ns)}
            for k, ei_local in enumerate(es):
                pos = ci * CH + k
                order[pos] = ids[ei_local]
                s[pos] = src[ids[ei_local]]
                ea[pos] = edge_attr[ids[ei_local]]
                t, j = divmod(k, 128)
                spT[j, ci * 4 + t, n_rank[int(dst[ids[ei_local]] - base)]] = 1.0
            for n, r in n_rank.items():
                vslot[n] = ci * MAXN + r

        idx_a = np.where(s < cfg.split, s + 1, 0)
        zb = cfg.tab_rows - 1 - cfg.viewb_off
        idx_b = np.where(s >= cfg.split, s + 1 - cfg.viewb_off, zb)
        d_loc = np.where(order >= 0, dst[order] - base, 0)
        d_gather = d_loc  # slab gather (pads -> row 0)

        e0T = np.zeros((64, cfg.e_pad), BF16)
        e0T[: cfg.nef, :] = ea.T.astype(BF16)

        deg = np.bincount(dst[ids] - base, minlength=cfg.nodes_pad).astype(np.float32)
        degT = deg.reshape(1, cfg.nodes_pad)

        x0slab = np.zeros((cfg.npc, 128), BF16)
        x0slab[:] = _pad_rows(x[base : base + cfg.npc], 128)
        x0T = np.zeros((64, cfg.nodes_pad), BF16)
        x0T[: cfg.nnf, : cfg.npc] = x[base : base + cfg.npc].T.astype(BF16)

        meta["orders"].append(order)
        in_maps.append(
            {
                "constf": constf,
                "constb": constb,
                "x0tab": x0tab,
                "x0slab": x0slab,
                "x0T": x0T,
                "e0T": e0T,
                "degT": degT,
                "spT": spT.reshape(128, nchunk * 4 * MAXN),
                "idxA": _wrap_idx(idx_a, cfg),
                "idxB": _wrap_idx(idx_b, cfg),
                "idxD": _wrap_idx(d_gather, cfg),
                "idxV": _wrap_idx(vslot, cfg, chunk=cfg.chunk),
            }
        )
    meta["f32_off"] = f32_off
    meta["bf_off"] = bf_off
    return cfg, in_maps, meta


# ---------------------------------------------------------------- bass builder


def build_program(cfg, f32_off, bf_off, n_layers=4, do_node=True, do_edge=True):
    import concourse.bass as bass
    import concourse.bacc as bacc
    import concourse.mybir as mybir
    import concourse.tile as tile

    dt = mybir.dt
    F32, BF, I16 = dt.float32, dt.bfloat16, dt.int16
    Alu = mybir.AluOpType
    Act = mybir.ActivationFunctionType
    AX = mybir.AxisListType

    nc = bacc.Bacc(
        "TRN2", target_bir_lowering=False, debug=False, num_devices=cfg.n_cores
    )
    H = cfg.hid
    ND = cfg.nchunk // 2
    NTAB = cfg.tab_rows

    # ---------------- dram params
    constf_d = nc.dram_tensor("constf", [128, sum(v[1] for v in f32_off.values())], F32, kind="ExternalInput")
    constb_d = nc.dram_tensor("constb", [128, sum(v[1] for v in bf_off.values())], BF, kind="ExternalInput")
    x0tab = nc.dram_tensor("x0tab", [NTAB, 128], BF, kind="ExternalInput")
    x0slab_d = nc.dram_tensor("x0slab", [cfg.npc, 128], BF, kind="ExternalInput")
    x0T_d = nc.dram_tensor("x0T", [64, cfg.nodes_pad], BF, kind="ExternalInput")
    e0T_d = nc.dram_tensor("e0T", [64, cfg.e_pad], BF, kind="ExternalInput")
    degT_d = nc.dram_tensor("degT", [1, cfg.nodes_pad], F32, kind="ExternalInput")
    spT_d = nc.dram_tensor("spT", [128, cfg.nchunk * 4 * 64], BF, kind="ExternalInput")
    idx_d = {}
    idx_cols = {
        "idxA": cfg.nchunk * (cfg.chunk // 16),
        "idxB": cfg.nchunk * (cfg.chunk // 16),
        "idxD": cfg.nchunk * (cfg.chunk // 16),
        "idxV": cfg.nodes_pad // 16,
    }
    for nm, ncol in idx_cols.items():
        idx_d[nm] = nc.dram_tensor(nm, [128, ncol], I16, kind="ExternalInput")

    out_node = nc.dram_tensor("out_node", [cfg.nodes_pad, cfg.nz], F32, kind="ExternalOutput")
    out_edge = nc.dram_tensor("out_edge", [1, cfg.e_pad], F32, kind="ExternalOutput")

    xtab = nc.dram_tensor("xtab", [NTAB, 128], BF, addr_space="Shared")
    slab = nc.dram_tensor("slab", [cfg.npc, 128], BF)
    eA = nc.dram_tensor("eA", [64, cfg.e_pad], BF)
    eB = nc.dram_tensor("eB", [64, cfg.e_pad], BF)
    parts = nc.dram_tensor("parts", [cfg.vrows, H], F32)

    # ---------------- persistent sbuf
    constf_s = nc.alloc_sbuf_tensor("constf_s", [128, constf_d.shape[1]], F32)
    constb_s = nc.alloc_sbuf_tensor("constb_s", [128, constb_d.shape[1]], BF)
    idx_s = {
        nm: nc.alloc_sbuf_tensor(nm + "_s", [128, idx_cols[nm]], I16) for nm in idx_d
    }
    degT_s = nc.alloc_sbuf_tensor("degT_s", [1, cfg.nodes_pad], F32)
    xT = [
        nc.alloc_sbuf_tensor("xT0", [64, cfg.nodes_pad], BF),
        nc.alloc_sbuf_tensor("xT1", [64, cfg.nodes_pad], BF),
    ]
    zt = nc.alloc_sbuf_tensor("zt", [128, 512], F32)
    ztb = nc.alloc_sbuf_tensor("ztb", [128, 128], BF)

    def cf(name):  # f32 const AP
        o, w, h = f32_off[name]
        return constf_s[0:h, o : o + w]

    def cb(name):
        o, w, h = bf_off[name]
        return constb_s[0:h, o : o + w]

    def cfp(name):  # full 128-partition view
        o, w, h = f32_off[name]
        return constf_s[:, o : o + w]

    def cbp(name):
        o, w, h = bf_off[name]
        return constb_s[:, o : o + w]

    with tile.TileContext(nc) as tc:
        sync = nc.sync
        gp = nc.gpsimd
        ve = nc.vector
        se = nc.scalar
        te = nc.tensor

        # ---- load persistents
        sync.dma_start(out=constf_s[:, :], in_=constf_d[:, :])
        sync.dma_start(out=constb_s[:, :], in_=constb_d[:, :])
        for nm in idx_d:
            sync.dma_start(out=idx_s[nm][:, :], in_=idx_d[nm][:, :])
        sync.dma_start(out=degT_s[:, :], in_=degT_d[:, :])
        sync.dma_start(out=xT[0][:, :], in_=x0T_d[:, :])
        ve.memset(zt[:, :], 0.0)
        ve.memset(ztb[:, :], 0.0)
        # zero rows of xtab (0 and last); zero row of partials table
        sync.dma_start(out=xtab[0:1, :], in_=ztb[0:1, :])
        sync.dma_start(out=xtab[NTAB - 1 : NTAB, :], in_=ztb[0:1, :])
        sync.dma_start(out=parts[cfg.vrows - 1 : cfg.vrows, :], in_=zt[0:1, 0:H])

        with (
            tc.tile_pool(name="ps_mm1", bufs=1, space="PSUM") as ps_mm1,
            tc.tile_pool(name="ps_nat", bufs=1, space="PSUM") as ps_nat,
            tc.tile_pool(name="ps_t2", bufs=2, space="PSUM") as ps_t2,
            tc.tile_pool(name="ps_mm2", bufs=1, space="PSUM") as ps_mm2,
            tc.tile_pool(name="ps_agg", bufs=1, space="PSUM") as ps_agg,
            tc.tile_pool(name="sb", bufs=3) as sb,
            tc.tile_pool(name="sb1", bufs=2) as sb1,
        ):
            CH = cfg.chunk  # 512
            for li in range(n_layers):
                nin, ein, eout, nout = cfg.layer_dims[li]
                tab = x0tab if li == 0 else xtab
                slab_src = x0slab_d if li == 0 else slab
                # edge stream: l0 reads host e0T, writes eA; then ping-pong
                eprev = (e0T_d, eA, eB, eA)[li]
                enext = (eA, eB, eA, None)[li]
                L = f"L{li}_"
                lastl = li == 3
                has_res = li in (1, 2)

                # ---------------- edge phase
                for d in range(ND if do_edge else 0):
                    g_t = []
                    for hh in range(2):
                        ci = 2 * d + hh
                        io = ci * (CH // 16)
                        q = (3 * ci) % 8
                        ga = sb.tile([128, 1, CH], BF, tag="ga")
                        gb = sb.tile([128, 1, CH], BF, tag="gb")
                        gd = sb.tile([128, 1, CH], BF, tag="gd")
                        gp.dma_gather(
                            ga[:, :, :], tab[0 : min(NTAB, 32768), :],
                            idx_s["idxA"][:, io : io + CH // 16],
                            num_idxs=CH, num_idxs_reg=CH, elem_size=128,
                            transpose=True,
                        )
                        gp.dma_gather(
                            gb[:, :, :], tab[cfg.viewb_off : NTAB, :],
                            idx_s["idxB"][:, io : io + CH // 16],
                            num_idxs=CH, num_idxs_reg=CH, elem_size=128,
                            transpose=True,
                        )
                        gp.dma_gather(
                            gd[:, :, :], slab_src[:, :],
                            idx_s["idxD"][:, io : io + CH // 16],
                            num_idxs=CH, num_idxs_reg=CH, elem_size=128,
                            transpose=True,
                        )
                        g_t.append((ga, gb, gd))

                    eTab = sb.tile([64, 2 * CH], BF, tag="eTab")
                    sync.dma_start(
                        out=eTab[:, :], in_=eprev[:, 2 * d * CH : (2 * d + 2) * CH]
                    )

                    h1T = ps_mm1.tile([64, 2 * CH], F32, tag="h1T")
                    for hh in range(2):
                        ga, gb, gd = g_t[hh]
                        co = CH * hh
                        te.matmul(
                            h1T[0:H, co : co + CH], cb(L + "Wsrc"), ga[0:nin, 0, :],
                            start=True, stop=False, tile_position=(0, 0),
                        )
                        te.matmul(
                            h1T[0:H, co : co + CH], cb(L + "Wsrc"), gb[0:nin, 0, :],
                            start=False, stop=False, tile_position=(0, 0),
                        )
                        te.matmul(
                            h1T[0:H, co : co + CH], cb(L + "Wdst"), gd[0:nin, 0, :],
                            start=False, stop=False, tile_position=(0, 0),
                        )
                        te.matmul(
                            h1T[0:H, co : co + CH],
                            cb(L + "We"),
                            eTab[0:ein, co : co + CH],
                            start=False, stop=True, tile_position=(0, 0),
                        )

                    h1Ts = sb.tile([64, 2 * CH], BF, tag="h1Ts")
                    se.activation(h1Ts[:, :], h1T[:, :], Act.Identity, bias=cf(L + "b1c"), scale=1.0)

                    nat = ps_nat.tile([128, 8, 64], BF, tag="nat")
                    for t in range(8):
                        te.transpose(
                            nat[:, t, :],
                            h1Ts[0:64, 128 * t : 128 * (t + 1)],
                            cbp("idB")[0:64, 0:64],
                            tile_position=(0, 0),
                        )

                    # evict natural to SBUF, then LN stats
                    nats = sb.tile([128, 8, 64], BF, tag="nats")
                    ve.tensor_copy(nats[:, :, :], nat[:, :, :])
                    ssum = sb.tile([128, 8], F32, tag="ssum")
                    sq = sb.tile([128, 8, 64], F32, tag="sq")
                    ssq = sb.tile([128, 8], F32, tag="ssq")
                    ve.tensor_reduce(ssum[:, :], nats[:, :, :], AX.X, Alu.add)
                    ve.tensor_tensor(sq[:, :, :], nats[:, :, :], nats[:, :, :], Alu.mult)
                    ve.tensor_reduce(ssq[:, :], sq[:, :, :], AX.X, Alu.add)
                    nmu = sb.tile([128, 8], F32, tag="nmu")
                    var = sb.tile([128, 8], F32, tag="var")
                    rs = sb.tile([128, 8], F32, tag="rs")
                    nmrs = sb.tile([128, 8], F32, tag="nmrs")
                    ve.tensor_scalar(nmu[:, :], ssum[:, :], -1.0 / 64, None, Alu.mult)
                    ve.tensor_scalar(var[:, :], ssq[:, :], 1.0 / 64, None, Alu.mult)
                    ve.tensor_tensor(sq[:, 0, 0:8], nmu[:, :], nmu[:, :], Alu.mult)
                    ve.tensor_tensor(var[:, :], var[:, :], sq[:, 0, 0:8], Alu.subtract)
                    se.activation(var[:, :], var[:, :], Act.Sqrt, bias=cfp("epsc"), scale=1.0)
                    ve.reciprocal(rs[:, :], var[:, :])
                    ve.tensor_tensor(nmrs[:, :], nmu[:, :], rs[:, :], Alu.mult)

                    # apply
                    t1 = sb.tile([128, 8, 64], F32, tag="t1")
                    for t in range(8):
                        se.activation(
                            t1[:, t, :], nats[:, t, :], Act.Identity,
                            bias=nmrs[:, t : t + 1], scale=rs[:, t : t + 1],
                        )
                    tmp = sb.tile([128, 8, 64], F32, tag="tmp")
                    hr = sb.tile([128, 8, 64], BF, tag="hr")
                    gb_ap = cfp(L + "gfull").unsqueeze(1).broadcast_to([128, 8, 64])
                    be_ap = cfp(L + "befull").unsqueeze(1).broadcast_to([128, 8, 64])
                    ve.tensor_tensor(tmp[:, :, :], t1[:, :, :], gb_ap, Alu.mult)
                    ve.tensor_tensor(tmp[:, :, :], tmp[:, :, :], be_ap, Alu.add)
                    ve.tensor_scalar(hr[:, :, :], tmp[:, :, :], 0.0, None, Alu.max)

                    # segment-sum partials via one-hot matmuls
                    spT_sb = sb.tile([128, 8, 64], BF, tag="spT_sb")
                    sync.dma_start(
                        out=spT_sb[:, :, :],
                        in_=spT_d[:, 2 * d * 4 * 64 : (2 * d + 2) * 4 * 64],
                    )
                    aggp = ps_agg.tile([64, 2, H], F32, tag="aggp")
                    for hh in range(2):
                        for t in range(4):
                            te.matmul(
                                aggp[:, hh, :],
                                spT_sb[:, 4 * hh + t, :],
                                hr[:, 4 * hh + t, :],
                                start=(t == 0), stop=(t == 3),
                                tile_position=(0, 0),
                            )
                    aggp_sb = sb.tile([64, 2, H], F32, tag="aggp_sb")
                    se.activation(aggp_sb[:, :, :], aggp[:, :, :], Act.Copy)
                    for hh in range(2):
                        ci = 2 * d + hh
                        sync.dma_start(
                            out=parts[ci * 64 : ci * 64 + 64, :], in_=aggp_sb[:, hh, :]
                        )

                    # transpose back for mm2 (transpose outs must be at psum partition 0)
                    hrT = ps_t2.tile([64, 2 * CH], BF, tag="hrT")
                    for t in range(8):
                        hh, tt_ = divmod(t, 4)
                        te.transpose(
                            hrT[0:64, CH * hh + 128 * tt_ : CH * hh + 128 * (tt_ + 1)],
                            hr[:, t, :],
                            cbp("idB")[:, 0:128],
                            tile_position=(0, 0),
                        )
                    hrTs = sb.tile([64, 2 * CH], BF, tag="hrTs")
                    ve.tensor_copy(hrTs[:, :], hrT[:, :])

                    enT = ps_mm2.tile([64, 2 * CH], F32, tag="enT")
                    for hh in range(2):
                        co = CH * hh
                        te.matmul(
                            enT[0:eout, co : co + CH],
                            cb(L + "W2"),
                            hrTs[0:H, co : co + CH],
                            start=True, stop=not has_res, tile_position=(0, 0),
                        )
                        if has_res:
                            te.matmul(
                                enT[0:eout, co : co + CH],
                                cbp("I64")[0:64, :],
                                eTab[0:64, co : co + CH],
                                start=False, stop=True, tile_position=(0, 0),
                            )
                    if not lastl:
                        enTs = sb.tile([64, 2 * CH], BF, tag="enTs")
                        se.activation(enTs[:, :], enT[:, :], Act.Identity, bias=cf(L + "b2c"), scale=1.0)
                        sync.dma_start(
                            out=enext[:, 2 * d * CH : (2 * d + 2) * CH], in_=enTs[:, :]
                        )
                    else:
                        eo = sb.tile([64, 2 * CH], F32, tag="enTs")
                        se.activation(
                            eo[0:1, :], enT[0:1, :], Act.Identity,
                            bias=cf(L + "b2c"), scale=1.0,
                        )
                        sync.dma_start(
                            out=out_edge[0:1, 2 * d * CH : (2 * d + 2) * CH], in_=eo[0:1, :]
                        )

                # ---------------- node phase
                xcur, xnxt = xT[li % 2], xT[(li + 1) % 2]
                NC_N = cfg.nodes_pad // CH
                for d in range(NC_N if do_node else 0):
                    c0 = d * CH
                    # gather per-node partial rows, transpose to [64, CH]
                    anat = sb1.tile([128, 4, H], F32, tag="anat")
                    gp.dma_gather(
                        anat[:, :, :], parts[:, :],
                        idx_s["idxV"][:, d * (CH // 16) : (d + 1) * (CH // 16)],
                        num_idxs=CH, num_idxs_reg=CH, elem_size=H, transpose=False,
                    )
                    aT = ps_t2.tile([128, CH], F32, tag="hrT")
                    for t in range(4):
                        te.transpose(
                            aT[0:64, 128 * t : 128 * (t + 1)], anat[:, t, :],
                            cfp("idT")[:, 0:128], tile_position=(0, 0),
                        )
                    aTs = sb1.tile([64, CH], F32, tag="aTs")
                    se.activation(aTs[:, :], aT[0:64, :], Act.Copy)

                    h1T = ps_mm1.tile([128, CH], F32, tag="h1T")
                    te.matmul(h1T[0:H, :], cb(L + "W1nx"), xcur[0:nin, c0 : c0 + CH],
                              start=True, stop=False, tile_position=(0, 0))
                    te.matmul(h1T[0:H, :], cf(L + "Cmat"), aTs[0:H, :],
                              start=False, stop=False, tile_position=(0, 0))
                    te.matmul(h1T[0:H, :], cf(L + "vbrow"), degT_s[0:1, c0 : c0 + CH],
                              start=False, stop=True, tile_position=(0, 0))

                    h1Ts = sb1.tile([64, CH], F32, tag="h1Ts_n")
                    se.activation(h1Ts[:, :], h1T[0:H, :], Act.Identity, bias=cf(L + "b1nc"), scale=1.0)

                    nat = ps_nat.tile([128, 4, 64], F32, tag="nat")
                    for t in range(4):
                        te.transpose(
                            nat[:, t, :], h1Ts[0:64, 128 * t : 128 * (t + 1)],
                            cfp("idT")[0:64, 0:64], tile_position=(0, 0),
                        )
                    nats = sb1.tile([128, 4, 64], F32, tag="nats_n")
                    ve.tensor_copy(nats[:, :, :], nat[:, :, :])
                    ssum = sb1.tile([128, 4], F32, tag="ssum_n")
                    sq = sb1.tile([128, 4, 64], F32, tag="sq_n")
                    ssq = sb1.tile([128, 4], F32, tag="ssq_n")
                    ve.tensor_reduce(ssum[:, :], nats[:, :, :], AX.X, Alu.add)
                    ve.tensor_tensor(sq[:, :, :], nats[:, :, :], nats[:, :, :], Alu.mult)
                    ve.tensor_reduce(ssq[:, :], sq[:, :, :], AX.X, Alu.add)
                    nmu = sb1.tile([128, 4], F32, tag="nmu_n")
                    var = sb1.tile([128, 4], F32, tag="var_n")
                    rs = sb1.tile([128, 4], F32, tag="rs_n")
                    nmrs = sb1.tile([128, 4], F32, tag="nmrs_n")
                    ve.tensor_scalar(nmu[:, :], ssum[:, :], -1.0 / 64, None, Alu.mult)
                    ve.tensor_scalar(var[:, :], ssq[:, :], 1.0 / 64, None, Alu.mult)
                    ve.tensor_tensor(sq[:, 0, 0:4], nmu[:, :], nmu[:, :], Alu.mult)
                    ve.tensor_tensor(var[:, :], var[:, :], sq[:, 0, 0:4], Alu.subtract)
                    se.activation(var[:, :], var[:, :], Act.Sqrt, bias=cfp("epsc"), scale=1.0)
                    ve.reciprocal(rs[:, :], var[:, :])
                    ve.tensor_tensor(nmrs[:, :], nmu[:, :], rs[:, :], Alu.mult)
                    t1 = sb1.tile([128, 4, 64], F32, tag="t1_n")
                    for t in range(4):
                        se.activation(
                            t1[:, t, :], nats[:, t, :], Act.Identity,
                            bias=nmrs[:, t : t + 1], scale=rs[:, t : t + 1],
                        )
                    hr = sb1.tile([128, 4, 64], F32, tag="hr_n")
                    gb_ap = cfp(L + "gnfull").unsqueeze(1).broadcast_to([128, 4, 64])
                    be_ap = cfp(L + "benfull").unsqueeze(1).broadcast_to([128, 4, 64])
                    ve.tensor_tensor(hr[:, :, :], t1[:, :, :], gb_ap, Alu.mult)
                    ve.tensor_tensor(hr[:, :, :], hr[:, :, :], be_ap, Alu.add)
                    ve.tensor_scalar(hr[:, :, :], hr[:, :, :], 0.0, None, Alu.max)
                    hrT = ps_t2.tile([128, CH], F32, tag="hrT")
                    for t in range(4):
                        te.transpose(
                            hrT[0:64, 128 * t : 128 * (t + 1)], hr[:, t, :],
                            cfp("idT")[:, 0:128], tile_position=(0, 0),
                        )
                    hrTs = sb1.tile([64, CH], F32, tag="hrTs_n")
                    ve.tensor_copy(hrTs[:, :], hrT[0:64, :])

                    xnT = ps_mm2.tile([128, CH], F32, tag="enT")
                    te.matmul(xnT[0:nout, :], cf(L + "W2n"), hrTs[0:H, :],
                              start=True, stop=not has_res, tile_position=(0, 0))
                    if has_res:
                        te.matmul(xnT[0:nout, :], cb("I64"), xcur[0:64, c0 : c0 + CH],
                                  start=False, stop=True, tile_position=(0, 0))

                    if not lastl:
                        se.activation(xnxt[0:64, c0 : c0 + CH], xnT[0:64, :], Act.Identity,
                                      bias=cf(L + "b2nc"), scale=1.0)
                        # natural bf16 slab rows
                        s_nat = ps_nat.tile([128, 4, 64], BF, tag="nat")
                        for t in range(4):
                            te.transpose(
                                s_nat[:, t, :], xnxt[0:64, c0 + 128 * t : c0 + 128 * (t + 1)],
                                cbp("idB")[0:64, 0:64], tile_position=(0, 0),
                            )
                        s_sb = sb1.tile([128, 4, 64], BF, tag="s_sb")
                        ve.tensor_copy(s_sb[:, :, :], s_nat[:, :, :])
                        for t in range(4):
                            r0 = c0 + 128 * t
                            if r0 >= cfg.npc:
                                continue
                            nrow = min(128, cfg.npc - r0)
                            sync.dma_start(out=slab[r0 : r0 + nrow, 0:64], in_=s_sb[0:nrow, t, :])
                            sync.dma_start(out=slab[r0 : r0 + nrow, 64:128], in_=s_sb[0:nrow, t, :])
                    else:
                        xo = sb1.tile([16, CH], F32, tag="hrTs_n")
                        se.activation(xo[:, :], xnT[0:16, :], Act.Identity,
                                      bias=cf(L + "b2nc"), scale=1.0)
                        o_nat = ps_nat.tile([128, 4, 16], F32, tag="nat")
                        for t in range(4):
                            te.transpose(
                                o_nat[:, t, :], xo[0:16, 128 * t : 128 * (t + 1)],
                                cfp("idT")[0:16, 0:16], tile_position=(0, 0),
                            )
                        o_sb = sb1.tile([128, 4, 16], F32, tag="s_sb")
                        ve.tensor_copy(o_sb[:, :, :], o_nat[:, :, :])
                        for t in range(4):
                            r0 = c0 + 128 * t
                            sync.dma_start(out=out_node[r0 : r0 + 128, :], in_=o_sb[:, t, :])

                if not lastl:
                    # allgather slabs into xtab rows [1, 1+n_nodes)
                    gp.collective_compute(
                        "AllGather",
                        Alu.bypass,
                        replica_groups=[list(range(cfg.n_cores))],
                        ins=[slab[:, :].opt()],
                        outs=[xtab[1 : 1 + cfg.n_nodes, :].opt()],
                    )

    nc.compile()
    return nc


# ---------------------------------------------------------------- runner

_CACHE = {}


def _get_program(cfg, meta):
    key = (cfg.n_nodes, cfg.e_pad)
    if key not in _CACHE:
        _CACHE[key] = build_program(cfg, meta["f32_off"], meta["bf_off"])
    return _CACHE[key]


def run_on_hw(cfg, in_maps, meta, trace=False):
    from concourse.bass_utils import run_bass_kernel_spmd

    nc = _get_program(cfg, meta)
    res = run_bass_kernel_spmd(
        nc, in_maps, core_ids=list(range(cfg.n_cores)), trace=trace
    )
    return nc, res


def unshard(cfg, meta, results):
    """results: list of per-core dicts with out_node/out_edge."""
    n_out = np.zeros((cfg.n_nodes, cfg.nz), np.float32)
    e_out = np.zeros((sum(meta["counts"]), 1), np.float32)
    for c in range(cfg.n_cores):
        r = results[c]
        n_out[c * cfg.npc : (c + 1) * cfg.npc] = r["out_node"][: cfg.npc]
        vals_flat = r["out_edge"].reshape(cfg.e_pad)
        order = meta["orders"][c]
        valid = order >= 0
        e_out[order[valid], 0] = vals_flat[valid]
    return n_out, e_out


def kernel(x, edge_index, edge_attr, params):
    cfg = Cfg()
    cfg, in_maps, meta = host_prep(cfg, x, edge_index, edge_attr, params)
    nc, res = run_on_hw(cfg, in_maps, meta, trace=False)
    n_out, e_out = unshard(cfg, meta, res.results)
    return n_out, e_out


# revision 41
# speedup vs baseline: 1.1624x; 1.1624x over previous
"""GNN message-passing encoder (4 layers) on 8 TRN2 NeuronCores via Bass/Tile.

Self-contained: host-side sharding/prep in numpy, SPMD bass program, gather of
full outputs. See test.py for the correctness/timing harness.
"""

import math
import os
import sys
from dataclasses import dataclass, field

import numpy as np

sys.path.insert(0, "/opt/trn_rl_repo")

import ml_dtypes  # noqa: E402

BF16 = ml_dtypes.bfloat16

# ---------------------------------------------------------------- config


@dataclass
class Cfg:
    n_nodes: int = 50000
    nnf: int = 16
    nef: int = 4
    hid: int = 64
    nz: int = 16
    n_cores: int = 8
    chunk: int = 512  # edges per chunk (half of a dual-chunk)
    ln_eps: float = 1e-5
    # derived
    npc: int = 0
    nodes_pad: int = 0
    agg_rows: int = 0
    tab_rows: int = 0
    viewb_off: int = 0
    split: int = 0
    nchunk: int = 0  # chunks per core (even)
    e_pad: int = 0
    ecols: int = 0
    vrows: int = 0  # agg partials table rows
    # (nin, ein, eout, nout) per layer
    layer_dims: tuple = ()

    def finalize(self, max_core_edges):
        assert self.n_nodes % self.n_cores == 0
        self.npc = self.n_nodes // self.n_cores
        self.nodes_pad = ((self.npc + self.chunk - 1) // self.chunk) * self.chunk
        self.agg_rows = self.nodes_pad + 128
        self.tab_rows = self.n_nodes + 2
        self.viewb_off = max(0, self.tab_rows - 32768)
        self.split = min(self.n_nodes, 32767)
        nd = (max_core_edges + 2 * self.chunk - 1) // (2 * self.chunk)
        nd = max(nd, 1)
        self.nchunk = 2 * nd
        self.e_pad = self.nchunk * self.chunk
        self.ecols = self.e_pad // 2
        h = self.hid
        self.layer_dims = (
            (self.nnf, self.nef, h, h),
            (h, h, h, h),
            (h, h, h, h),
            (h, h, 1, self.nz),
        )
        return self


# ---------------------------------------------------------------- host prep


def _wrap_idx(flat, cfg, chunk=None):
    """[n*chunk] int -> [128, n*(chunk//16)] int16, per-chunk wrapped (i -> [i%16, i//16])."""
    ch = chunk or cfg.chunk
    a = np.asarray(flat, np.int64).reshape(-1, ch)
    n = a.shape[0]
    a = a.reshape(n, ch // 16, 16).transpose(0, 2, 1)  # [n,16,ch/16]
    a = a.transpose(1, 0, 2).reshape(16, n * (ch // 16))
    assert a.max() <= 32767 and a.min() >= 0, (a.min(), a.max())
    return np.tile(a, (8, 1)).astype(np.int16)


def _pad_rows(x, width):
    """[n, f] -> [n, width] bf16 (zero pad cols)."""
    n, f = x.shape
    out = np.zeros((n, width), BF16)
    out[:, :f] = x.astype(BF16)
    return out


def host_prep(cfg_base, x, edge_index, edge_attr, params):
    """Shard + build all per-core input arrays. Returns (cfg, in_maps, meta)."""
    x = np.asarray(x, np.float32)
    edge_index = np.asarray(edge_index, np.int64)
    edge_attr = np.asarray(edge_attr, np.float32)
    src, dst = edge_index[0], edge_index[1]
    E = src.shape[0]

    npc = cfg_base.n_nodes // cfg_base.n_cores
    core_of = dst // npc
    perms, counts = [], []
    for c in range(cfg_base.n_cores):
        ids = np.nonzero(core_of == c)[0]
        ids = ids[np.argsort(dst[ids], kind="stable")]
        perms.append(ids)
        counts.append(len(ids))
    cfg = cfg_base
    cfg.finalize(max(counts))

    # ---- weights prep (shared across cores)
    H = cfg.hid
    WL = []  # per-layer dict of arrays
    for li, p in enumerate(params):
        nin, ein, eout, nout = cfg.layer_dims[li]
        pe, pn = p["edge"], p["node"]
        W1 = np.asarray(pe["W1"], np.float32)
        w = {}
        w["Wsrc"] = W1[:nin].astype(BF16)  # [nin, H]
        w["Wdst"] = W1[nin : 2 * nin].astype(BF16)
        w["We"] = W1[2 * nin :].astype(BF16)  # [ein, H]
        b1 = np.asarray(pe["b1"], np.float32)
        w["b1c"] = b1.reshape(H, 1)
        w["gfull"] = np.tile(np.asarray(pe["g"], np.float32), (128, 1))
        w["befull"] = np.tile(np.asarray(pe["be"], np.float32), (128, 1))
        W2 = np.asarray(pe["W2"], np.float32)  # [H, eout]
        w["W2"] = W2.astype(BF16)
        b2 = np.asarray(pe["b2"], np.float32)
        w["b2c"] = b2.reshape(eout, 1)
        # node side
        W1n = np.asarray(pn["W1"], np.float32)
        w["W1nx"] = W1n[:nin].astype(BF16)  # [nin, H]
        W1na = W1n[nin:]  # [eout, H]
        w["Cmat"] = (W2 @ W1na).astype(np.float32)  # [H, H]
        w["vbrow"] = (b2 @ W1na).reshape(1, H).astype(np.float32)
        w["b1nc"] = np.asarray(pn["b1"], np.float32).reshape(H, 1)
        w["gnfull"] = np.tile(np.asarray(pn["g"], np.float32), (128, 1))
        w["benfull"] = np.tile(np.asarray(pn["be"], np.float32), (128, 1))
        w["W2n"] = np.asarray(pn["W2"], np.float32)  # [H, nout]
        w["b2nc"] = np.asarray(pn["b2"], np.float32).reshape(nout, 1)
        WL.append(w)

    # pack f32 consts into one [128, *] tensor, bf16 consts into another
    f32_cols, bf_cols = [], []
    f32_off, bf_off = {}, {}

    def put(cols, off, name, arr, dt):
        a = np.zeros((128, arr.shape[1]), dt)
        a[: arr.shape[0]] = arr.astype(dt)
        off[name] = (sum(c.shape[1] for c in cols), arr.shape[1], arr.shape[0])
        cols.append(a)

    ident = np.eye(128, dtype=np.float32)
    put(f32_cols, f32_off, "idT", ident, np.float32)
    put(bf_cols, bf_off, "idB", ident, BF16)
    put(bf_cols, bf_off, "I64", np.eye(64, dtype=np.float32), BF16)
    put(f32_cols, f32_off, "epsc", np.full((128, 1), cfg.ln_eps, np.float32), np.float32)
    for li, w in enumerate(WL):
        for k, v in w.items():
            if v.dtype == BF16:
                put(bf_cols, bf_off, f"L{li}_{k}", v, BF16)
            else:
                put(f32_cols, f32_off, f"L{li}_{k}", v, np.float32)
    constf = np.concatenate(f32_cols, axis=1)
    constb = np.concatenate(bf_cols, axis=1)

    # ---- global gather table for layer 1 (built from x0)
    x0tab = np.zeros((cfg.tab_rows, 128), BF16)
    x0tab[1 : 1 + cfg.n_nodes] = _pad_rows(x, 128)

    # ---- per-core chunking: whole-dst-segment chunks (<=CHUNK edges, <=64 nodes)
    CH = cfg.chunk
    MAXN = 64
    core_chunks = []  # per core: list of (edge_ids list, node_ids list)
    for c in range(cfg.n_cores):
        ids = perms[c]
        base = c * cfg.npc
        dl = dst[ids] - base
        # group boundaries by dst value
        chunks = []
        cur_e, cur_n = [], []
        i = 0
        Ec = len(ids)
        while i < Ec:
            j = i
            while j < Ec and dl[j] == dl[i]:
                j += 1
            seg = list(range(i, j))
            if cur_e and (len(cur_e) + len(seg) > CH or len(cur_n) >= MAXN):
                chunks.append((cur_e, cur_n))
                cur_e, cur_n = [], []
            # a single segment larger than CH cannot happen (max degree << CH)
            assert len(seg) <= CH
            cur_e += seg
            cur_n.append(int(dl[i]))
            i = j
        if cur_e:
            chunks.append((cur_e, cur_n))
        core_chunks.append(chunks)

    nchunk = max(len(ch) for ch in core_chunks)
    nchunk = ((nchunk + 1) // 2) * 2  # even for dual-chunks
    cfg.nchunk = nchunk
    cfg.e_pad = nchunk * CH
    cfg.ecols = cfg.e_pad // 2
    cfg.vrows = nchunk * MAXN + 1  # partials table rows (+1 zero row)
    assert cfg.vrows - 1 <= 32767

    in_maps = []
    meta = {"perms": perms, "counts": counts, "cfg": cfg, "orders": []}
    for c in range(cfg.n_cores):
        ids = perms[c]
        base = c * cfg.npc
        chunks = core_chunks[c]
        order = np.full(cfg.e_pad, -1, np.int64)  # flat pos -> orig edge id
        s = np.zeros(cfg.e_pad, np.int64)
        ea = np.zeros((cfg.e_pad, cfg.nef), np.float32)
        spT = np.zeros((128, nchunk * 4, MAXN), BF16)
        vslot = np.full(cfg.nodes_pad, cfg.vrows - 1, np.int64)  # default zero row
        for ci, (es, ns) in enumerate(chunks):
            n_rank = {n: r for r, n in enumerate(ns)}
            for k, ei_local in enumerate(es):
                pos = ci * CH + k
                order[pos] = ids[ei_local]
                s[pos] = src[ids[ei_local]]
                ea[pos] = edge_attr[ids[ei_local]]
                t, j = divmod(k, 128)
                spT[j, ci * 4 + t, n_rank[int(dst[ids[ei_local]] - base)]] = 1.0
            for n, r in n_rank.items():
                vslot[n] = ci * MAXN + r

        idx_a = np.where(s < cfg.split, s + 1, 0)
        zb = cfg.tab_rows - 1 - cfg.viewb_off
        idx_b = np.where(s >= cfg.split, s + 1 - cfg.viewb_off, zb)
        d_loc = np.where(order >= 0, dst[order] - base, 0)
        d_gather = d_loc  # slab gather (pads -> row 0)

        e0T = np.zeros((64, cfg.e_pad), BF16)
        e0T[: cfg.nef, :] = ea.T.astype(BF16)

        deg = np.bincount(dst[ids] - base, minlength=cfg.nodes_pad).astype(np.float32)
        degT = deg.reshape(1, cfg.nodes_pad)

        x0slab = np.zeros((cfg.npc, 128), BF16)
        x0slab[:] = _pad_rows(x[base : base + cfg.npc], 128)
        x0T = np.zeros((64, cfg.nodes_pad), BF16)
        x0T[: cfg.nnf, : cfg.npc] = x[base : base + cfg.npc].T.astype(BF16)

        meta["orders"].append(order)
        in_maps.append(
            {
                "constf": constf,
                "constb": constb,
                "x0tab": x0tab,
                "x0slab": x0slab,
                "x0T": x0T,
                "e0T": e0T,
                "degT": degT,
                "spT": spT.reshape(128, nchunk * 4 * MAXN),
                "idxA": _wrap_idx(idx_a, cfg),
                "idxB": _wrap_idx(idx_b, cfg),
                "idxD": _wrap_idx(d_gather, cfg),
                "idxV": _wrap_idx(vslot, cfg, chunk=cfg.chunk),
            }
        )
    meta["f32_off"] = f32_off
    meta["bf_off"] = bf_off
    return cfg, in_maps, meta


# ---------------------------------------------------------------- bass builder


def build_program(cfg, f32_off, bf_off, n_layers=4, do_node=True, do_edge=True):
    import concourse.bass as bass
    import concourse.bacc as bacc
    import concourse.mybir as mybir
    import concourse.tile as tile

    dt = mybir.dt
    F32, BF, I16 = dt.float32, dt.bfloat16, dt.int16
    Alu = mybir.AluOpType
    Act = mybir.ActivationFunctionType
    AX = mybir.AxisListType

    nc = bacc.Bacc(
        "TRN2", target_bir_lowering=False, debug=False, num_devices=cfg.n_cores,
    )
    H = cfg.hid
    ND = cfg.nchunk // 2
    NTAB = cfg.tab_rows

    # ---------------- dram params
    constf_d = nc.dram_tensor("constf", [128, sum(v[1] for v in f32_off.values())], F32, kind="ExternalInput")
    constb_d = nc.dram_tensor("constb", [128, sum(v[1] for v in bf_off.values())], BF, kind="ExternalInput")
    x0tab = nc.dram_tensor("x0tab", [NTAB, 128], BF, kind="ExternalInput")
    x0slab_d = nc.dram_tensor("x0slab", [cfg.npc, 128], BF, kind="ExternalInput")
    x0T_d = nc.dram_tensor("x0T", [64, cfg.nodes_pad], BF, kind="ExternalInput")
    e0T_d = nc.dram_tensor("e0T", [64, cfg.e_pad], BF, kind="ExternalInput")
    degT_d = nc.dram_tensor("degT", [1, cfg.nodes_pad], F32, kind="ExternalInput")
    spT_d = nc.dram_tensor("spT", [128, cfg.nchunk * 4 * 64], BF, kind="ExternalInput")
    idx_d = {}
    idx_cols = {
        "idxA": cfg.nchunk * (cfg.chunk // 16),
        "idxB": cfg.nchunk * (cfg.chunk // 16),
        "idxD": cfg.nchunk * (cfg.chunk // 16),
        "idxV": cfg.nodes_pad // 16,
    }
    for nm, ncol in idx_cols.items():
        idx_d[nm] = nc.dram_tensor(nm, [128, ncol], I16, kind="ExternalInput")

    out_node = nc.dram_tensor("out_node", [cfg.nodes_pad, cfg.nz], F32, kind="ExternalOutput")
    out_edge = nc.dram_tensor("out_edge", [1, cfg.e_pad], F32, kind="ExternalOutput")

    xtab = nc.dram_tensor("xtab", [NTAB, 128], BF, addr_space="Shared")
    slab = nc.dram_tensor("slab", [cfg.npc, 128], BF)
    eA = nc.dram_tensor("eA", [64, cfg.e_pad], BF)
    eB = nc.dram_tensor("eB", [64, cfg.e_pad], BF)
    parts = nc.dram_tensor("parts", [cfg.vrows, H], F32)

    # ---------------- persistent sbuf
    constf_s = nc.alloc_sbuf_tensor("constf_s", [128, constf_d.shape[1]], F32)
    constb_s = nc.alloc_sbuf_tensor("constb_s", [128, constb_d.shape[1]], BF)
    idx_s = {
        nm: nc.alloc_sbuf_tensor(nm + "_s", [128, idx_cols[nm]], I16) for nm in idx_d
    }
    degT_s = nc.alloc_sbuf_tensor("degT_s", [1, cfg.nodes_pad], F32)
    xT = [
        nc.alloc_sbuf_tensor("xT0", [64, cfg.nodes_pad], BF),
        nc.alloc_sbuf_tensor("xT1", [64, cfg.nodes_pad], BF),
    ]
    zt = nc.alloc_sbuf_tensor("zt", [128, 512], F32)
    ztb = nc.alloc_sbuf_tensor("ztb", [128, 128], BF)

    def cf(name):  # f32 const AP
        o, w, h = f32_off[name]
        return constf_s[0:h, o : o + w]

    def cb(name):
        o, w, h = bf_off[name]
        return constb_s[0:h, o : o + w]

    def cfp(name):  # full 128-partition view
        o, w, h = f32_off[name]
        return constf_s[:, o : o + w]

    def cbp(name):
        o, w, h = bf_off[name]
        return constb_s[:, o : o + w]

    with tile.TileContext(nc) as tc:
        sync = nc.sync
        gp = nc.gpsimd
        ve = nc.vector
        se = nc.scalar
        te = nc.tensor

        # ---- load persistents
        sync.dma_start(out=constf_s[:, :], in_=constf_d[:, :])
        sync.dma_start(out=constb_s[:, :], in_=constb_d[:, :])
        for nm in idx_d:
            sync.dma_start(out=idx_s[nm][:, :], in_=idx_d[nm][:, :])
        sync.dma_start(out=degT_s[:, :], in_=degT_d[:, :])
        sync.dma_start(out=xT[0][:, :], in_=x0T_d[:, :])
        ve.memset(zt[:, :], 0.0)
        ve.memset(ztb[:, :], 0.0)
        # zero rows of xtab (0 and last); zero row of partials table
        sync.dma_start(out=xtab[0:1, :], in_=ztb[0:1, :])
        sync.dma_start(out=xtab[NTAB - 1 : NTAB, :], in_=ztb[0:1, :])
        sync.dma_start(out=parts[cfg.vrows - 1 : cfg.vrows, :], in_=zt[0:1, 0:H])

        with (
            tc.tile_pool(name="ps_mm1", bufs=1, space="PSUM") as ps_mm1,
            tc.tile_pool(name="ps_nat", bufs=1, space="PSUM") as ps_nat,
            tc.tile_pool(name="ps_t2", bufs=2, space="PSUM") as ps_t2,
            tc.tile_pool(name="ps_mm2", bufs=1, space="PSUM") as ps_mm2,
            tc.tile_pool(name="ps_agg", bufs=1, space="PSUM") as ps_agg,
            tc.tile_pool(name="sb", bufs=3) as sb,
            tc.tile_pool(name="sb1", bufs=2) as sb1,
        ):
            CH = cfg.chunk  # 512
            for li in range(n_layers):
                nin, ein, eout, nout = cfg.layer_dims[li]
                tab = x0tab if li == 0 else xtab
                slab_src = x0slab_d if li == 0 else slab
                # edge stream: l0 reads host e0T, writes eA; then ping-pong
                eprev = (e0T_d, eA, eB, eA)[li]
                enext = (eA, eB, eA, None)[li]
                L = f"L{li}_"
                lastl = li == 3
                has_res = li in (1, 2)

                # ---------------- edge phase
                for d in range(ND if do_edge else 0):
                    g_t = []
                    for hh in range(2):
                        ci = 2 * d + hh
                        io = ci * (CH // 16)
                        q = (3 * ci) % 8
                        ga = sb.tile([128, 1, CH], BF, tag="ga")
                        gb = sb.tile([128, 1, CH], BF, tag="gb")
                        gd = sb.tile([128, 1, CH], BF, tag="gd")
                        gp.dma_gather(
                            ga[:, :, :], tab[0 : min(NTAB, 32768), :],
                            idx_s["idxA"][:, io : io + CH // 16],
                            num_idxs=CH, num_idxs_reg=CH, elem_size=128,
                            transpose=True,
                        )
                        gp.dma_gather(
                            gb[:, :, :], tab[cfg.viewb_off : NTAB, :],
                            idx_s["idxB"][:, io : io + CH // 16],
                            num_idxs=CH, num_idxs_reg=CH, elem_size=128,
                            transpose=True,
                        )
                        gp.dma_gather(
                            gd[:, :, :], slab_src[:, :],
                            idx_s["idxD"][:, io : io + CH // 16],
                            num_idxs=CH, num_idxs_reg=CH, elem_size=128,
                            transpose=True,
                        )
                        g_t.append((ga, gb, gd))

                    eTab = sb.tile([64, 2 * CH], BF, tag="eTab")
                    sync.dma_start(
                        out=eTab[:, :], in_=eprev[:, 2 * d * CH : (2 * d + 2) * CH]
                    )

                    h1T = ps_mm1.tile([64, 2 * CH], F32, tag="h1T")
                    for hh in range(2):
                        ga, gb, gd = g_t[hh]
                        co = CH * hh
                        te.matmul(
                            h1T[0:H, co : co + CH], cb(L + "Wsrc"), ga[0:nin, 0, :],
                            start=True, stop=False, tile_position=(0, 0),
                        )
                        te.matmul(
                            h1T[0:H, co : co + CH], cb(L + "Wsrc"), gb[0:nin, 0, :],
                            start=False, stop=False, tile_position=(0, 0),
                        )
                        te.matmul(
                            h1T[0:H, co : co + CH], cb(L + "Wdst"), gd[0:nin, 0, :],
                            start=False, stop=False, tile_position=(0, 0),
                        )
                        te.matmul(
                            h1T[0:H, co : co + CH],
                            cb(L + "We"),
                            eTab[0:ein, co : co + CH],
                            start=False, stop=True, tile_position=(0, 0),
                        )

                    h1Ts = sb.tile([64, 2 * CH], BF, tag="h1Ts")
                    se.activation(h1Ts[:, :], h1T[:, :], Act.Identity, bias=cf(L + "b1c"), scale=1.0)

                    nat = ps_nat.tile([128, 8, 64], BF, tag="nat")
                    for t in range(8):
                        te.transpose(
                            nat[:, t, :],
                            h1Ts[0:64, 128 * t : 128 * (t + 1)],
                            cbp("idB")[0:64, 0:64],
                            tile_position=(0, 0),
                        )

                    # evict natural to SBUF, then LN stats
                    nats = sb.tile([128, 8, 64], BF, tag="nats")
                    ve.tensor_copy(nats[:, :, :], nat[:, :, :])
                    ssum = sb.tile([128, 8], F32, tag="ssum")
                    sq = sb.tile([128, 8, 64], F32, tag="sq")
                    ssq = sb.tile([128, 8], F32, tag="ssq")
                    ve.tensor_reduce(ssum[:, :], nats[:, :, :], AX.X, Alu.add)
                    ve.tensor_tensor(sq[:, :, :], nats[:, :, :], nats[:, :, :], Alu.mult)
                    ve.tensor_reduce(ssq[:, :], sq[:, :, :], AX.X, Alu.add)
                    nmu = sb.tile([128, 8], F32, tag="nmu")
                    var = sb.tile([128, 8], F32, tag="var")
                    rs = sb.tile([128, 8], F32, tag="rs")
                    nmrs = sb.tile([128, 8], F32, tag="nmrs")
                    ve.tensor_scalar(nmu[:, :], ssum[:, :], -1.0 / 64, None, Alu.mult)
                    ve.tensor_scalar(var[:, :], ssq[:, :], 1.0 / 64, None, Alu.mult)
                    ve.tensor_tensor(sq[:, 0, 0:8], nmu[:, :], nmu[:, :], Alu.mult)
                    ve.tensor_tensor(var[:, :], var[:, :], sq[:, 0, 0:8], Alu.subtract)
                    se.activation(var[:, :], var[:, :], Act.Sqrt, bias=cfp("epsc"), scale=1.0)
                    ve.reciprocal(rs[:, :], var[:, :])
                    ve.tensor_tensor(nmrs[:, :], nmu[:, :], rs[:, :], Alu.mult)

                    # apply
                    t1 = sb.tile([128, 8, 64], F32, tag="t1")
                    for t in range(8):
                        se.activation(
                            t1[:, t, :], nats[:, t, :], Act.Identity,
                            bias=nmrs[:, t : t + 1], scale=rs[:, t : t + 1],
                        )
                    tmp = sb.tile([128, 8, 64], F32, tag="tmp")
                    hr = sb.tile([128, 8, 64], BF, tag="hr")
                    gb_ap = cfp(L + "gfull").unsqueeze(1).broadcast_to([128, 8, 64])
                    be_ap = cfp(L + "befull").unsqueeze(1).broadcast_to([128, 8, 64])
                    ve.tensor_tensor(tmp[:, :, :], t1[:, :, :], gb_ap, Alu.mult)
                    ve.tensor_tensor(tmp[:, :, :], tmp[:, :, :], be_ap, Alu.add)
                    ve.tensor_scalar(hr[:, :, :], tmp[:, :, :], 0.0, None, Alu.max)

                    # segment-sum partials via one-hot matmuls
                    spT_sb = sb.tile([128, 8, 64], BF, tag="spT_sb")
                    sync.dma_start(
                        out=spT_sb[:, :, :],
                        in_=spT_d[:, 2 * d * 4 * 64 : (2 * d + 2) * 4 * 64],
                    )
                    aggp = ps_agg.tile([64, 2, H], F32, tag="aggp")
                    for hh in range(2):
                        for t in range(4):
                            te.matmul(
                                aggp[:, hh, :],
                                spT_sb[:, 4 * hh + t, :],
                                hr[:, 4 * hh + t, :],
                                start=(t == 0), stop=(t == 3),
                                tile_position=(0, 0),
                            )
                    aggp_sb = sb.tile([64, 2, H], F32, tag="aggp_sb")
                    se.activation(aggp_sb[:, :, :], aggp[:, :, :], Act.Copy)
                    for hh in range(2):
                        ci = 2 * d + hh
                        sync.dma_start(
                            out=parts[ci * 64 : ci * 64 + 64, :], in_=aggp_sb[:, hh, :]
                        )

                    # transpose back for mm2 (transpose outs must be at psum partition 0)
                    hrT = ps_t2.tile([64, 2 * CH], BF, tag="hrT")
                    for t in range(8):
                        hh, tt_ = divmod(t, 4)
                        te.transpose(
                            hrT[0:64, CH * hh + 128 * tt_ : CH * hh + 128 * (tt_ + 1)],
                            hr[:, t, :],
                            cbp("idB")[:, 0:128],
                            tile_position=(0, 0),
                        )
                    hrTs = sb.tile([64, 2 * CH], BF, tag="hrTs")
                    ve.tensor_copy(hrTs[:, :], hrT[:, :])

                    enT = ps_mm2.tile([64, 2 * CH], F32, tag="enT")
                    for hh in range(2):
                        co = CH * hh
                        te.matmul(
                            enT[0:eout, co : co + CH],
                            cb(L + "W2"),
                            hrTs[0:H, co : co + CH],
                            start=True, stop=not has_res, tile_position=(0, 0),
                        )
                        if has_res:
                            te.matmul(
                                enT[0:eout, co : co + CH],
                                cbp("I64")[0:64, :],
                                eTab[0:64, co : co + CH],
                                start=False, stop=True, tile_position=(0, 0),
                            )
                    if not lastl:
                        enTs = sb.tile([64, 2 * CH], BF, tag="enTs")
                        se.activation(enTs[:, :], enT[:, :], Act.Identity, bias=cf(L + "b2c"), scale=1.0)
                        sync.dma_start(
                            out=enext[:, 2 * d * CH : (2 * d + 2) * CH], in_=enTs[:, :]
                        )
                    else:
                        eo = sb.tile([64, 2 * CH], F32, tag="enTs")
                        se.activation(
                            eo[0:1, :], enT[0:1, :], Act.Identity,
                            bias=cf(L + "b2c"), scale=1.0,
                        )
                        sync.dma_start(
                            out=out_edge[0:1, 2 * d * CH : (2 * d + 2) * CH], in_=eo[0:1, :]
                        )

                # ---------------- node phase
                xcur, xnxt = xT[li % 2], xT[(li + 1) % 2]
                NC_N = cfg.nodes_pad // CH
                for d in range(NC_N if do_node else 0):
                    c0 = d * CH
                    # gather per-node partial rows, transpose to [64, CH]
                    anat = sb1.tile([128, 4, H], F32, tag="anat")
                    gp.dma_gather(
                        anat[:, :, :], parts[:, :],
                        idx_s["idxV"][:, d * (CH // 16) : (d + 1) * (CH // 16)],
                        num_idxs=CH, num_idxs_reg=CH, elem_size=H, transpose=False,
                    )
                    aT = ps_t2.tile([128, CH], F32, tag="hrT")
                    for t in range(4):
                        te.transpose(
                            aT[0:64, 128 * t : 128 * (t + 1)], anat[:, t, :],
                            cfp("idT")[:, 0:128], tile_position=(0, 0),
                        )
                    aTs = sb1.tile([64, CH], F32, tag="aTs")
                    se.activation(aTs[:, :], aT[0:64, :], Act.Copy)

                    h1T = ps_mm1.tile([128, CH], F32, tag="h1T")
                    te.matmul(h1T[0:H, :], cb(L + "W1nx"), xcur[0:nin, c0 : c0 + CH],
                              start=True, stop=False, tile_position=(0, 0))
                    te.matmul(h1T[0:H, :], cf(L + "Cmat"), aTs[0:H, :],
                              start=False, stop=False, tile_position=(0, 0))
                    te.matmul(h1T[0:H, :], cf(L + "vbrow"), degT_s[0:1, c0 : c0 + CH],
                              start=False, stop=True, tile_position=(0, 0))

                    h1Ts = sb1.tile([64, CH], F32, tag="h1Ts_n")
                    se.activation(h1Ts[:, :], h1T[0:H, :], Act.Identity, bias=cf(L + "b1nc"), scale=1.0)

                    nat = ps_nat.tile([128, 4, 64], F32, tag="nat")
                    for t in range(4):
                        te.transpose(
                            nat[:, t, :], h1Ts[0:64, 128 * t : 128 * (t + 1)],
                            cfp("idT")[0:64, 0:64], tile_position=(0, 0),
                        )
                    nats = sb1.tile([128, 4, 64], F32, tag="nats_n")
                    ve.tensor_copy(nats[:, :, :], nat[:, :, :])
                    ssum = sb1.tile([128, 4], F32, tag="ssum_n")
                    sq = sb1.tile([128, 4, 64], F32, tag="sq_n")
                    ssq = sb1.tile([128, 4], F32, tag="ssq_n")
                    ve.tensor_reduce(ssum[:, :], nats[:, :, :], AX.X, Alu.add)
                    ve.tensor_tensor(sq[:, :, :], nats[:, :, :], nats[:, :, :], Alu.mult)
                    ve.tensor_reduce(ssq[:, :], sq[:, :, :], AX.X, Alu.add)
                    nmu = sb1.tile([128, 4], F32, tag="nmu_n")
                    var = sb1.tile([128, 4], F32, tag="var_n")
                    rs = sb1.tile([128, 4], F32, tag="rs_n")
                    nmrs = sb1.tile([128, 4], F32, tag="nmrs_n")
                    ve.tensor_scalar(nmu[:, :], ssum[:, :], -1.0 / 64, None, Alu.mult)
                    ve.tensor_scalar(var[:, :], ssq[:, :], 1.0 / 64, None, Alu.mult)
                    ve.tensor_tensor(sq[:, 0, 0:4], nmu[:, :], nmu[:, :], Alu.mult)
                    ve.tensor_tensor(var[:, :], var[:, :], sq[:, 0, 0:4], Alu.subtract)
                    se.activation(var[:, :], var[:, :], Act.Sqrt, bias=cfp("epsc"), scale=1.0)
                    ve.reciprocal(rs[:, :], var[:, :])
                    ve.tensor_tensor(nmrs[:, :], nmu[:, :], rs[:, :], Alu.mult)
                    t1 = sb1.tile([128, 4, 64], F32, tag="t1_n")
                    for t in range(4):
                        se.activation(
                            t1[:, t, :], nats[:, t, :], Act.Identity,
                            bias=nmrs[:, t : t + 1], scale=rs[:, t : t + 1],
                        )
                    hr = sb1.tile([128, 4, 64], F32, tag="hr_n")
                    gb_ap = cfp(L + "gnfull").unsqueeze(1).broadcast_to([128, 4, 64])
                    be_ap = cfp(L + "benfull").unsqueeze(1).broadcast_to([128, 4, 64])
                    ve.tensor_tensor(hr[:, :, :], t1[:, :, :], gb_ap, Alu.mult)
                    ve.tensor_tensor(hr[:, :, :], hr[:, :, :], be_ap, Alu.add)
                    ve.tensor_scalar(hr[:, :, :], hr[:, :, :], 0.0, None, Alu.max)
                    hrT = ps_t2.tile([128, CH], F32, tag="hrT")
                    for t in range(4):
                        te.transpose(
                            hrT[0:64, 128 * t : 128 * (t + 1)], hr[:, t, :],
                            cfp("idT")[:, 0:128], tile_position=(0, 0),
                        )
                    hrTs = sb1.tile([64, CH], F32, tag="hrTs_n")
                    ve.tensor_copy(hrTs[:, :], hrT[0:64, :])

                    xnT = ps_mm2.tile([128, CH], F32, tag="enT")
                    te.matmul(xnT[0:nout, :], cf(L + "W2n"), hrTs[0:H, :],
                              start=True, stop=not has_res, tile_position=(0, 0))
                    if has_res:
                        te.matmul(xnT[0:nout, :], cb("I64"), xcur[0:64, c0 : c0 + CH],
                                  start=False, stop=True, tile_position=(0, 0))

                    if not lastl:
                        se.activation(xnxt[0:64, c0 : c0 + CH], xnT[0:64, :], Act.Identity,
                                      bias=cf(L + "b2nc"), scale=1.0)
                        # natural bf16 slab rows
                        s_nat = ps_nat.tile([128, 4, 64], BF, tag="nat")
                        for t in range(4):
                            te.transpose(
                                s_nat[:, t, :], xnxt[0:64, c0 + 128 * t : c0 + 128 * (t + 1)],
                                cbp("idB")[0:64, 0:64], tile_position=(0, 0),
                            )
                        s_sb = sb1.tile([128, 4, 64], BF, tag="s_sb")
                        ve.tensor_copy(s_sb[:, :, :], s_nat[:, :, :])
                        for t in range(4):
                            r0 = c0 + 128 * t
                            if r0 >= cfg.npc:
                                continue
                            nrow = min(128, cfg.npc - r0)
                            sync.dma_start(out=slab[r0 : r0 + nrow, 0:64], in_=s_sb[0:nrow, t, :])
                            sync.dma_start(out=slab[r0 : r0 + nrow, 64:128], in_=s_sb[0:nrow, t, :])
                    else:
                        xo = sb1.tile([16, CH], F32, tag="hrTs_n")
                        se.activation(xo[:, :], xnT[0:16, :], Act.Identity,
                                      bias=cf(L + "b2nc"), scale=1.0)
                        o_nat = ps_nat.tile([128, 4, 16], F32, tag="nat")
                        for t in range(4):
                            te.transpose(
                                o_nat[:, t, :], xo[0:16, 128 * t : 128 * (t + 1)],
                                cfp("idT")[0:16, 0:16], tile_position=(0, 0),
                            )
                        o_sb = sb1.tile([128, 4, 16], F32, tag="s_sb")
                        ve.tensor_copy(o_sb[:, :, :], o_nat[:, :, :])
                        for t in range(4):
                            r0 = c0 + 128 * t
                            sync.dma_start(out=out_node[r0 : r0 + 128, :], in_=o_sb[:, t, :])

                if not lastl:
                    # allgather slabs into xtab rows [1, 1+n_nodes)
                    gp.collective_compute(
                        "AllGather",
                        Alu.bypass,
                        replica_groups=[list(range(cfg.n_cores))],
                        ins=[slab[:, :].opt()],
                        outs=[xtab[1 : 1 + cfg.n_nodes, :].opt()],
                    )

    nc.compile()
    return nc


# ---------------------------------------------------------------- runner

_CACHE = {}


def _get_program(cfg, meta):
    key = (cfg.n_nodes, cfg.e_pad)
    if key not in _CACHE:
        _CACHE[key] = build_program(cfg, meta["f32_off"], meta["bf_off"])
    return _CACHE[key]


def run_on_hw(cfg, in_maps, meta, trace=False):
    from concourse.bass_utils import run_bass_kernel_spmd

    nc = _get_program(cfg, meta)
    res = run_bass_kernel_spmd(
        nc, in_maps, core_ids=list(range(cfg.n_cores)), trace=trace
    )
    return nc, res


def unshard(cfg, meta, results):
    """results: list of per-core dicts with out_node/out_edge."""
    n_out = np.zeros((cfg.n_nodes, cfg.nz), np.float32)
    e_out = np.zeros((sum(meta["counts"]), 1), np.float32)
    for c in range(cfg.n_cores):
        r = results[c]
        n_out[c * cfg.npc : (c + 1) * cfg.npc] = r["out_node"][: cfg.npc]
        vals_flat = r["out_edge"].reshape(cfg.e_pad)
        order = meta["orders"][c]
        valid = order >= 0
        e_out[order[valid], 0] = vals_flat[valid]
    return n_out, e_out


def kernel(x, edge_index, edge_attr, params):
    cfg = Cfg()
    cfg, in_maps, meta = host_prep(cfg, x, edge_index, edge_attr, params)
    nc, res = run_on_hw(cfg, in_maps, meta, trace=False)
    n_out, e_out = unshard(cfg, meta, res.results)
    return n_out, e_out
